# revision 1
# baseline (speedup 1.0000x reference)
"""CycleConsistencyLoss on 8 Trainium2 NeuronCores (Bass/Tile, SPMD data-parallel).

Math (per batch, clip [M,D], sent [N,D], prefix masks):
  soft_nn(src,tgt): w = softmax_j(-dist(src_i,tgt_j) masked); nn = w @ tgt
  dist = (|s|^2+|t|^2-2 s.t)/D; softmax shift-invariance =>
  w[i,j] prop exp((2 s_i.t_j - |t_j|^2)/D) * mask_j  (row terms cancel)
  index_nn = sum_u u*beta / sum_u beta over tgt2 = src embeddings
  loss_c = mean_b sum_i (index_nn[i]-i)^2 * mask_i / len_b

Device pipeline per (slot, cycle):
  A: dots[t, i] = X^T Y on PE; Et = exp(2/D dots + bias_t) on ACT (1024-wide;
     bias folds -|t|^2/D and -1e4*(1-mask): exp underflows to exact 0)
     nn_num[d,i] += Xn[tb]^T Et ; den[i] += ones^T Et (thin mm, psum row 32)
  C: nns = nn_num * bcast(1/den)  (approx recip + GPSIMD partition_broadcast)
  D: dots2[u,i] = Y[:,ub]^T nns ; Bt = exp(...); [den;num_hi;num_lo] += thin mm
  final: index_nn = num/den; per-(slot,cycle) loss rows -> DRAM; host averages.

Per-slot length specialization: batches sorted by size, slot gets 8 similar
batches across cores; block/chunk counts from the slot max lens (SPMD-safe).
"""
import os
import sys

sys.path.insert(0, "/opt/trn_rl_repo")

import numpy as np
import ml_dtypes

import concourse.bass as bass
import concourse.tile as tile
from concourse import bacc, mybir
from concourse.bass_utils import run_bass_kernel_spmd

F32 = mybir.dt.float32
F32R = mybir.dt.float32r
BF16 = mybir.dt.bfloat16
EXP = mybir.ActivationFunctionType.Exp
ALU = mybir.AluOpType

B, M, N, D = 32, 1024, 1024, 128
NCORES = 8
SLOTS = B // NCORES  # 4
PEN = -1.0e4  # exp(PEN + x) == 0.0 exactly in fp32
USE_BF16 = os.environ.get("CCL_F32R") != "1"  # bf16 matmuls by default

_PROGRAM_CACHE = {}
LAST_RESULT = None


def _plan_chunks(nblk):
    """Split nblk*128 extent into (offset, width) chunks: [512, rest>=256]."""
    ext = nblk * 128
    if ext <= 512:
        return [(0, 512)]
    return [(0, 512), (512, max(256, ext - 512))]


def _emit(nc, tc, ctx, io, plans):
    ts = bass.ts
    scale = 2.0 / D
    DT = BF16 if USE_BF16 else F32R

    const = ctx.enter_context(tc.tile_pool(name="const", bufs=1))
    emb = ctx.enter_context(tc.tile_pool(name="emb", bufs=2))
    etp = ctx.enter_context(tc.tile_pool(name="etp", bufs=6))
    nnp = ctx.enter_context(tc.tile_pool(name="nnp", bufs=4))
    bcp = ctx.enter_context(tc.tile_pool(name="bcp", bufs=2))
    rrp = ctx.enter_context(tc.tile_pool(name="rrp", bufs=2))
    fin = ctx.enter_context(tc.tile_pool(name="fin", bufs=1))

    ps_mm = ctx.enter_context(tc.tile_pool(name="ps_mm", bufs=3, space="PSUM"))
    ps_nn = ctx.enter_context(tc.tile_pool(name="ps_nn", bufs=2, space="PSUM"))
    ps_dn = ctx.enter_context(tc.tile_pool(name="ps_dn", bufs=1, space="PSUM"))
    ps_th = ctx.enter_context(tc.tile_pool(name="ps_th", bufs=2, space="PSUM"))

    thin_w = const.tile([128, M // 128, 3], DT, tag="thin_w")
    nc.sync.dma_start(out=thin_w, in_=io["thin_w"])
    iota_t = const.tile([2 * SLOTS, 2, 512], F32, tag="iota")
    nc.sync.dma_start(out=iota_t, in_=io["iota8"].rearrange("r (q x) -> r q x", q=2))
    masks_t = const.tile([2 * SLOTS, 2, 512], F32, tag="masks")
    nc.sync.dma_start(out=masks_t, in_=io["masks8"].rearrange("r (q x) -> r q x", q=2))
    rlens_t = const.tile([2 * SLOTS, 1], F32, tag="rlens")
    nc.sync.dma_start(out=rlens_t, in_=io["rlens"])

    # staging: [role(den,hi,lo), k, q, x]; memset 1.0 covers never-written cols
    th_sb = fin.tile([3, 2 * SLOTS, 2, 512], F32, tag="th_sb")
    nc.vector.memset(th_sb, 1.0)

    # ---- slot embedding tiles (lazy, emb pool bufs=2 prefetches) ----
    slot_tiles = {}

    def get_slot(s):
        if s in slot_tiles:
            return slot_tiles[s]
        t = {}
        t["ct"] = emb.tile([128, M], DT, tag="ct", name=f"ct{s}")
        nc.sync.dma_start(out=t["ct"], in_=io["cembT"][s])
        t["st"] = emb.tile([128, N], DT, tag="st", name=f"st{s}")
        nc.sync.dma_start(out=t["st"], in_=io["sembT"][s])
        t["cn"] = emb.tile([128, M // 128, D], DT, tag="cn", name=f"cn{s}")
        nc.sync.dma_start(out=t["cn"], in_=io["cembN"][s])
        t["sn"] = emb.tile([128, N // 128, D], DT, tag="sn", name=f"sn{s}")
        nc.sync.dma_start(out=t["sn"], in_=io["sembN"][s])
        t["bias_c"] = emb.tile([128, M // 128], F32, tag="bias_c", name=f"bc{s}")
        nc.sync.dma_start(out=t["bias_c"], in_=io["bias_c"][s])
        t["bias_s"] = emb.tile([128, M // 128], F32, tag="bias_s", name=f"bs{s}")
        nc.sync.dma_start(out=t["bias_s"], in_=io["bias_s"][s])
        slot_tiles[s] = t
        return t

    # ---- build unit list: one unit per (slot, cycle, chunk) ----
    units = []
    for s in range(SLOTS):
        cb, sb = plans[s]
        for c in range(2):
            n_tb = sb if c == 0 else cb
            n_ub = cb if c == 0 else sb
            for q, (off, w) in enumerate(_plan_chunks(n_ub)):
                units.append(dict(s=s, c=c, q=q, off=off, w=w,
                                  n_tb=n_tb, n_ub=n_ub, k=2 * s + c))
    pairs = [units[i:i + 2] for i in range(0, len(units), 2)]

    def a_iter(u, tb, et):
        t = get_slot(u["s"])
        X = t["st"] if u["c"] == 0 else t["ct"]
        Y = t["ct"] if u["c"] == 0 else t["st"]
        Xn = t["sn"] if u["c"] == 0 else t["cn"]
        b_tgt = t["bias_s"] if u["c"] == 0 else t["bias_c"]
        w, off = u["w"], u["off"]
        j = u["j"]
        mm = ps_mm.tile([128, 512], F32, tag="mm", name=f"mmA_{u['k']}_{u['q']}_{tb}")
        nc.tensor.matmul(mm[:, 0:w], lhsT=X[:, ts(tb, 128)],
                         rhs=Y[:, off:off + w], start=True, stop=True)
        nc.scalar.activation(et[:, 0:w], mm[:, 0:w], EXP,
                             bias=b_tgt[:, tb:tb + 1], scale=scale)
        first, last = tb == 0, tb == u["n_tb"] - 1
        nc.tensor.matmul(u["dn"][32 * j:32 * j + 1, 0:w],
                         lhsT=thin_w[:, tb, 0:1], rhs=et[:, 0:w],
                         start=first, stop=last)
        nc.tensor.matmul(u["nn"][:, 0:w], lhsT=Xn[:, tb, :], rhs=et[:, 0:w],
                         start=first, stop=last)

    def c_phase(u):
        w, j = u["w"], u["j"]
        dnc = rrp.tile([1, 512], F32, tag="dnc")
        nc.vector.tensor_copy(dnc[:, 0:w], u["dn"][32 * j:32 * j + 1, 0:w])
        rr = rrp.tile([1, 512], F32, tag="rr")
        nc.vector.reciprocal_approx_fast(out=rr[:, 0:w], in_=dnc[:, 0:w])
        bc = bcp.tile([128, 512], F32, tag="bc")
        nc.gpsimd.partition_broadcast(bc[:, 0:w], rr[:, 0:w])
        nt = nnp.tile([128, 512], DT, tag="nns")
        nc.vector.scalar_tensor_tensor(nt[:, 0:w], in0=u["nn"][:, 0:w],
                                       scalar=1.0, in1=bc[:, 0:w],
                                       op0=ALU.bypass, op1=ALU.mult)
        u["nns"] = nt

    def d_iter(u, ub, bt):
        t = get_slot(u["s"])
        Y = t["ct"] if u["c"] == 0 else t["st"]
        b_src = t["bias_c"] if u["c"] == 0 else t["bias_s"]
        w, j = u["w"], u["j"]
        mm2 = ps_mm.tile([128, 512], F32, tag="mm", name=f"mmD_{u['k']}_{u['q']}_{ub}")
        nc.tensor.matmul(mm2[:, 0:w], lhsT=Y[:, ts(ub, 128)],
                         rhs=u["nns"][:, 0:w], start=True, stop=True)
        nc.scalar.activation(bt[:, 0:w], mm2[:, 0:w], EXP,
                             bias=b_src[:, ub:ub + 1], scale=scale)
        nc.tensor.matmul(u["th"][64 * j:64 * j + 3, 0:w], lhsT=thin_w[:, ub, :],
                         rhs=bt[:, 0:w], start=(ub == 0), stop=(ub == u["n_ub"] - 1))
        if ub == u["n_ub"] - 1:
            nc.vector.tensor_copy(th_sb[:, u["k"], u["q"], 0:w],
                                  u["th"][64 * j:64 * j + 3, 0:w])

    def segment(d_units, a_units, pi):
        if a_units:
            dn = ps_dn.tile([33, 512], F32, tag="dn", name=f"dn_{pi}")
            for j, u in enumerate(a_units):
                u["j"], u["dn"] = j, dn
                u["nn"] = ps_nn.tile([128, 512], F32, tag="nn",
                                     name=f"nn_{u['k']}_{u['q']}")
        if d_units:
            th = ps_th.tile([67, 512], F32, tag="th", name=f"th_{pi}")
            for j, u in enumerate(d_units):
                u["j"], u["th"] = j, th
        n_iter = max([u["n_tb"] for u in a_units] + [u["n_ub"] for u in d_units]
                     + [0])
        for i in range(n_iter):
            for u in a_units:
                if i < u["n_tb"]:
                    et = etp.tile([128, 512], DT, tag="et")
                    a_iter(u, i, et)
            for u in d_units:
                if i < u["n_ub"]:
                    bt = etp.tile([128, 512], DT, tag="et")
                    d_iter(u, i, bt)

    prev = []
    for pi, pair in enumerate(pairs):
        segment(prev, pair, pi)
        for u in pair:
            c_phase(u)
        prev = pair
    segment(prev, [], len(pairs))

    # ---- final ----
    den8 = fin.tile([2 * SLOTS, 2, 512], F32, tag="den8")
    hi8 = fin.tile([2 * SLOTS, 2, 512], F32, tag="hi8")
    lo8 = fin.tile([2 * SLOTS, 2, 512], F32, tag="lo8")
    nc.sync.dma_start(out=den8, in_=th_sb[0:1, :, :, :])
    nc.sync.dma_start(out=hi8, in_=th_sb[1:2, :, :, :])
    nc.sync.dma_start(out=lo8, in_=th_sb[2:3, :, :, :])
    num8 = fin.tile([2 * SLOTS, 2, 512], F32, tag="num8")
    nc.vector.tensor_add(num8, hi8, lo8)
    rden = fin.tile([2 * SLOTS, 2, 512], F32, tag="rden")
    scr = fin.tile([2 * SLOTS, 2, 512], F32, tag="scr")
    nc.vector.reciprocal_approx_accurate(out=rden, in_=den8, scratch=scr)
    idx = fin.tile([2 * SLOTS, 2, 512], F32, tag="idx")
    nc.vector.tensor_mul(idx, num8, rden)
    ierr = fin.tile([2 * SLOTS, 2, 512], F32, tag="ierr")
    nc.vector.tensor_sub(ierr, idx, iota_t)
    tmp = fin.tile([2 * SLOTS, 2, 512], F32, tag="tmp")
    nc.vector.tensor_mul(tmp, ierr, masks_t)
    sq = fin.tile([2 * SLOTS, 2, 512], F32, tag="sq")
    sums = fin.tile([2 * SLOTS, 1], F32, tag="sums")
    nc.vector.scalar_tensor_tensor(sq, in0=tmp, scalar=1.0, in1=ierr,
                                   op0=ALU.bypass, op1=ALU.mult, accum_out=sums)
    loss = fin.tile([2 * SLOTS, 1], F32, tag="loss")
    nc.vector.tensor_mul(loss, sums, rlens_t)
    nc.sync.dma_start(out=io["loss8"], in_=loss)


def _build_program(plans):
    key = (USE_BF16, tuple(plans))
    if key in _PROGRAM_CACHE:
        return _PROGRAM_CACHE[key]
    nc = bacc.Bacc("TRN2", target_bir_lowering=False, debug=False,
                   num_devices=NCORES)
    NB = M // 128
    DT = BF16 if USE_BF16 else F32R
    io = {
        "cembT": nc.dram_tensor("cembT", [SLOTS, D, M], DT, kind="ExternalInput").ap(),
        "sembT": nc.dram_tensor("sembT", [SLOTS, D, N], DT, kind="ExternalInput").ap(),
        "cembN": nc.dram_tensor("cembN", [SLOTS, 128, NB, D], DT, kind="ExternalInput").ap(),
        "sembN": nc.dram_tensor("sembN", [SLOTS, 128, NB, D], DT, kind="ExternalInput").ap(),
        "bias_c": nc.dram_tensor("bias_c", [SLOTS, 128, NB], F32, kind="ExternalInput").ap(),
        "bias_s": nc.dram_tensor("bias_s", [SLOTS, 128, NB], F32, kind="ExternalInput").ap(),
        "thin_w": nc.dram_tensor("thin_w", [128, NB, 3], DT, kind="ExternalInput").ap(),
        "iota8": nc.dram_tensor("iota8", [2 * SLOTS, M], F32, kind="ExternalInput").ap(),
        "masks8": nc.dram_tensor("masks8", [2 * SLOTS, M], F32, kind="ExternalInput").ap(),
        "rlens": nc.dram_tensor("rlens", [2 * SLOTS, 1], F32, kind="ExternalInput").ap(),
        "loss8": nc.dram_tensor("loss8", [2 * SLOTS, 1], F32, kind="ExternalOutput").ap(),
    }
    from contextlib import ExitStack
    with tile.TileContext(nc) as tc:
        with ExitStack() as ctx:
            _emit(nc, tc, ctx, io, plans)
    nc.compile()
    _PROGRAM_CACHE[key] = nc
    return nc


def _host_prep(clip_emb, clip_mask, clip_lens, sent_emb, sent_mask, sent_lens):
    """Sorted batch->$(core,slot) assignment, per-slot plans, per-core inputs."""
    NB = M // 128
    mdt = ml_dtypes.bfloat16 if USE_BF16 else np.float32

    cb_all = np.ceil(clip_lens / 128).astype(int)
    sb_all = np.ceil(sent_lens / 128).astype(int)
    order = np.argsort(-(cb_all + sb_all) * 1000 - cb_all)  # big batches first
    plans = []
    assign = {}  # (core, slot) -> batch
    for s in range(SLOTS):
        grp = order[8 * s:8 * s + 8]
        plans.append((int(cb_all[grp].max()), int(sb_all[grp].max())))
        for core, b in enumerate(grp):
            assign[(core, s)] = int(b)

    sq_c = np.einsum("bmd,bmd->bm", clip_emb, clip_emb)
    sq_s = np.einsum("bnd,bnd->bn", sent_emb, sent_emb)
    bias_c = (-sq_c / D + PEN * (1.0 - clip_mask)).astype(np.float32)
    bias_s = (-sq_s / D + PEN * (1.0 - sent_mask)).astype(np.float32)

    thin_w = np.zeros((128, NB, 3), np.float32)
    thin_w[:, :, 0] = 1.0
    u = np.arange(128)[:, None] + 128 * np.arange(NB)[None, :]
    thin_w[:, :, 1] = (u & ~3).astype(np.float32)   # exact in bf16 (8-bit mantissa)
    thin_w[:, :, 2] = (u & 3).astype(np.float32)
    iota8 = np.broadcast_to(np.arange(M, dtype=np.float32), (2 * SLOTS, M)).copy()

    in_maps = []
    for core in range(NCORES):
        bs = [assign[(core, s)] for s in range(SLOTS)]
        ce = clip_emb[bs]
        se = sent_emb[bs]
        masks8 = np.empty((2 * SLOTS, M), np.float32)
        rlens = np.empty((2 * SLOTS, 1), np.float32)
        for s, b in enumerate(bs):
            masks8[2 * s + 0] = clip_mask[b]
            masks8[2 * s + 1] = sent_mask[b]
            rlens[2 * s + 0] = 1.0 / clip_lens[b]
            rlens[2 * s + 1] = 1.0 / sent_lens[b]
        in_maps.append({
            "cembT": np.ascontiguousarray(ce.transpose(0, 2, 1)).astype(mdt),
            "sembT": np.ascontiguousarray(se.transpose(0, 2, 1)).astype(mdt),
            "cembN": np.ascontiguousarray(
                ce.reshape(SLOTS, NB, 128, D).transpose(0, 2, 1, 3)).astype(mdt),
            "sembN": np.ascontiguousarray(
                se.reshape(SLOTS, NB, 128, D).transpose(0, 2, 1, 3)).astype(mdt),
            "bias_c": np.ascontiguousarray(
                bias_c[bs].reshape(SLOTS, NB, 128).transpose(0, 2, 1)),
            "bias_s": np.ascontiguousarray(
                bias_s[bs].reshape(SLOTS, NB, 128).transpose(0, 2, 1)),
            "thin_w": thin_w.astype(mdt),
            "iota8": iota8,
            "masks8": masks8,
            "rlens": rlens,
        })
    return in_maps, assign, plans


def kernel(clip_emb, clip_mask, clip_lens, sent_emb, sent_mask, sent_lens):
    global LAST_RESULT
    clip_emb = np.asarray(clip_emb, np.float32)
    sent_emb = np.asarray(sent_emb, np.float32)
    clip_mask = np.asarray(clip_mask, np.float32)
    sent_mask = np.asarray(sent_mask, np.float32)
    clip_lens = np.asarray(clip_lens, np.float32)
    sent_lens = np.asarray(sent_lens, np.float32)

    in_maps, _, plans = _host_prep(clip_emb, clip_mask, clip_lens,
                                   sent_emb, sent_mask, sent_lens)
    nc = _build_program(plans)
    res = run_bass_kernel_spmd(nc, in_maps, list(range(NCORES)))
    LAST_RESULT = res

    rows = np.stack([res.results[c]["loss8"].reshape(2 * SLOTS) for c in range(NCORES)])
    clip_loss = rows[:, 0::2].mean()
    sent_loss = rows[:, 1::2].mean()
    return (np.float32(clip_loss), np.float32(sent_loss))

